# revision 8
# baseline (speedup 1.0000x reference)
"""Trainium2 Bass kernel for nn_CustomLSTM (B=64, S=512, D=H=1024) on 8 cores.

Sharding: hidden-dimension split (128 hidden units / core), full batch on
every core.  Per timestep each core computes its 512 gate rows
(i,f,o,g x 128) as [128 gates, 64 batch] tiles (hidden-major), updates its
c/h slices, and broadcasts its h slice [128, 64] bf16 to all 8 cores'
SBUF via remote_dma_broadcast (slot = sender core id).  The input
projection xg = x @ Wi.T + (bi+bh) is computed on-device in a first phase
(each core computes its own 512 gate columns for all (t, b)) and streamed
from DRAM during the recurrence, folded into PSUM with an identity-matmul
transpose-accumulate.

kernel(**inputs) takes the full unsharded inputs and returns
(hidden_seq, (h_final, c_final)) matching the reference.
"""

import numpy as np
import ml_dtypes

B, S, D, H = 64, 512, 1024, 1024
NCORES = 8
HS = H // NCORES      # 128 hidden units per core
GS = 4 * HS           # 512 gate rows per core
P = 128               # partitions
NB = B                # 64 batch columns per tile
KC = D // P           # 8 contraction chunks

_BF = ml_dtypes.bfloat16


def build_nc(seq_len=S, comm=True):
    import concourse.bass as bass
    import concourse.mybir as mybir
    from concourse import bacc
    from contextlib import ExitStack

    f32 = mybir.dt.float32
    bf16 = mybir.dt.bfloat16
    SIG = mybir.ActivationFunctionType.Sigmoid
    TANH = mybir.ActivationFunctionType.Tanh
    MULT = mybir.AluOpType.mult
    ADD = mybir.AluOpType.add

    R = seq_len * B // P  # projection row-chunks (128 (t,b) rows each)
    assert seq_len >= 4 and seq_len % 2 == 0

    nc = bacc.Bacc("TRN2", target_bir_lowering=False, debug=False,
                   num_devices=NCORES)

    # ---- DRAM ----
    xT = nc.dram_tensor("xT", [D, seq_len * B], bf16, kind="ExternalInput")
    wiT = nc.dram_tensor("wiT", [D, GS], bf16, kind="ExternalInput")
    whT = nc.dram_tensor("whT", [D, GS], bf16, kind="ExternalInput")
    biasb = nc.dram_tensor("biasb", [P, GS], f32, kind="ExternalInput")
    h0T = nc.dram_tensor("h0T", [P, NCORES * NB], bf16, kind="ExternalInput")
    c0 = nc.dram_tensor("c0", [P, NB], f32, kind="ExternalInput")
    eye = nc.dram_tensor("eye", [NB, NB], bf16, kind="ExternalInput")
    hsT = nc.dram_tensor("hsT", [seq_len * P, NB], f32, kind="ExternalOutput")
    cT = nc.dram_tensor("cT", [P, NB], f32, kind="ExternalOutput")
    xg = nc.dram_tensor("xg", [seq_len * B, GS], bf16)  # internal

    es = ExitStack()
    with es:
        # ---- SBUF ----
        whT_sb = es.enter_context(nc.sbuf_tensor("whT_sb", [P, KC * GS], bf16))
        wiT_sb = es.enter_context(nc.sbuf_tensor("wiT_sb", [P, KC * GS], bf16))
        biasb_sb = es.enter_context(nc.sbuf_tensor("biasb_sb", [P, GS], f32))
        xt0 = es.enter_context(nc.sbuf_tensor("xt0", [P, KC * P], bf16))
        xt1 = es.enter_context(nc.sbuf_tensor("xt1", [P, KC * P], bf16))
        pst0 = es.enter_context(nc.sbuf_tensor("pst0", [P, GS], bf16))
        pst1 = es.enter_context(nc.sbuf_tensor("pst1", [P, GS], bf16))
        xg0 = es.enter_context(nc.sbuf_tensor("xg0", [NB, GS], bf16))
        xg1 = es.enter_context(nc.sbuf_tensor("xg1", [NB, GS], bf16))
        hb0 = es.enter_context(nc.sbuf_tensor("hb0", [P, NCORES * NB], bf16))
        hb1 = es.enter_context(nc.sbuf_tensor("hb1", [P, NCORES * NB], bf16))
        hst0 = es.enter_context(nc.sbuf_tensor("hst0", [P, NB], bf16))
        hst1 = es.enter_context(nc.sbuf_tensor("hst1", [P, NB], bf16))
        hf0 = es.enter_context(nc.sbuf_tensor("hf0", [P, NB], f32))
        hf1 = es.enter_context(nc.sbuf_tensor("hf1", [P, NB], f32))
        ga0 = es.enter_context(nc.sbuf_tensor("ga0", [P, 4 * NB], f32))
        ga1 = es.enter_context(nc.sbuf_tensor("ga1", [P, 4 * NB], f32))
        tc0 = es.enter_context(nc.sbuf_tensor("tc0", [P, NB], f32))
        tc1 = es.enter_context(nc.sbuf_tensor("tc1", [P, NB], f32))
        tmp_ig = es.enter_context(nc.sbuf_tensor("tmp_ig", [P, NB], f32))
        c_sb = es.enter_context(nc.sbuf_tensor("c_sb", [P, NB], f32))
        eye_sb = es.enter_context(nc.sbuf_tensor("eye_sb", [NB, NB], bf16))
        # ---- PSUM ----
        psp0 = es.enter_context(nc.psum_tensor("psp0", [P, GS], f32))
        psp1 = es.enter_context(nc.psum_tensor("psp1", [P, GS], f32))
        psr0 = es.enter_context(nc.psum_tensor("psr0", [P, 4 * NB], f32))
        psr1 = es.enter_context(nc.psum_tensor("psr1", [P, 4 * NB], f32))

        xts = [xt0, xt1]
        psts = [pst0, pst1]
        xgs = [xg0, xg1]
        hbs = [hb0, hb1]
        hsts = [hst0, hst1]
        hfs = [hf0, hf1]
        gas = [ga0, ga1]
        tcs = [tc0, tc1]
        psps = [psp0, psp1]
        psrs = [psr0, psr1]

        # ---- semaphores ----
        s_w = es.enter_context(nc.semaphore("s_w"))
        s_xt = [es.enter_context(nc.semaphore(f"s_xt{i}")) for i in range(2)]
        s_pp = es.enter_context(nc.semaphore("s_pp"))
        s_pc = es.enter_context(nc.semaphore("s_pc"))
        s_xgw = [es.enter_context(nc.semaphore(f"s_xgw{i}")) for i in range(2)]
        s_xgr = [es.enter_context(nc.semaphore(f"s_xgr{i}")) for i in range(2)]
        s_hrs = [es.enter_context(nc.semaphore(f"s_hr{i}")) for i in range(2)]
        s_bloc = [es.enter_context(nc.semaphore(f"s_bloc{i}")) for i in range(2)]
        s_prep = es.enter_context(nc.semaphore("s_prep"))
        s_mm = es.enter_context(nc.semaphore("s_mm"))
        s_act = es.enter_context(nc.semaphore("s_act"))
        s_dve = es.enter_context(nc.semaphore("s_dve"))
        s_hso = [es.enter_context(nc.semaphore(f"s_hso{i}")) for i in range(2)]
        s_fin = es.enter_context(nc.semaphore("s_fin"))

        N_INIT_DMA = 6  # whT, wiT, biasb, h0T, c0, eye
        block = es.enter_context(nc.Block())

        @block.sync
        def _(sync):
            # init loads
            sync.dma_start(
                whT_sb[:, :].rearrange("p (c g) -> p c g", c=KC),
                whT[:, :].rearrange("(c p) g -> p c g", p=P),
            ).then_inc(s_w, 16)
            sync.dma_start(
                wiT_sb[:, :].rearrange("p (c g) -> p c g", c=KC),
                wiT[:, :].rearrange("(c p) g -> p c g", p=P),
            ).then_inc(s_w, 16)
            sync.dma_start(biasb_sb[:, :], biasb[:, :]).then_inc(s_w, 16)
            sync.dma_start(hb0[:, :], h0T[:, :]).then_inc(s_w, 16)
            sync.dma_start(c_sb[:, :], c0[:, :]).then_inc(s_w, 16)
            sync.dma_start(eye_sb[:, :], eye[:, :]).then_inc(s_w, 16)

            # projection xT prologue (row-chunks 0, 1)
            for i in range(2):
                sync.dma_start(
                    xts[i][:, :].rearrange("p (c r) -> p c r", c=KC),
                    xT[:, i * P:(i + 1) * P].rearrange("(c p) r -> p c r", p=P),
                ).then_inc(s_xt[i], 16)
            # projection loop
            for rc in range(R):
                sync.wait_ge(s_pc, rc + 1)
                sync.dma_start(
                    xg[rc * P:(rc + 1) * P, :], psts[rc % 2][:, :]
                ).then_inc(s_xgw[rc % 2], 16)
                if rc + 2 < R:
                    sync.wait_ge(s_pp, rc + 1)
                    sync.dma_start(
                        xts[rc % 2][:, :].rearrange("p (c r) -> p c r", c=KC),
                        xT[:, (rc + 2) * P:(rc + 3) * P].rearrange(
                            "(c p) r -> p c r", p=P
                        ),
                    ).then_inc(s_xt[rc % 2], 16)
            # recurrence xg prologue (t = 0, 1)
            for t in range(2):
                rcs = t // 2  # source row-chunk
                sync.wait_ge(s_xgw[rcs % 2], 16 * (rcs // 2 + 1))
                sync.dma_start(
                    xgs[t % 2][:, :], xg[t * B:(t + 1) * B, :]
                ).then_inc(s_xgr[t % 2], 16)
            # recurrence loop
            for t in range(seq_len):
                sync.wait_ge(s_dve, 5 * t + 5)
                sync.dma_start(
                    hsT[t * P:(t + 1) * P, :], hfs[t % 2][:, :]
                ).then_inc(s_hso[t % 2], 16)
                if t + 2 < seq_len:
                    rcs = (t + 2) // 2
                    sync.wait_ge(s_xgw[rcs % 2], 16 * (rcs // 2 + 1))
                    sync.wait_ge(s_mm, t + 1)
                    sync.dma_start(
                        xgs[t % 2][:, :], xg[(t + 2) * B:(t + 3) * B, :]
                    ).then_inc(s_xgr[t % 2], 16)
            # final c
            sync.wait_ge(s_dve, 5 * seq_len)
            sync.dma_start(cT[:, :], c_sb[:, :]).then_inc(s_fin, 16)
            sync.wait_ge(s_fin, 16)

        @block.gpsimd
        def _(gpsimd: bass.BassGpSimd):
            if not comm:
                return
            pid = gpsimd.partition_id()
            for t in range(seq_len):
                hbn = hbs[(t + 1) % 2]
                gpsimd.remote_dma_broadcast(
                    out_ap=hbn[:, bass.ts(pid, NB)],
                    in_ap=hsts[t % 2][:, :],
                    remote_sem=s_hrs[(t + 1) % 2],
                    local_sem=s_bloc[t % 2],
                    rdests=[(0, k) for k in range(NCORES)],
                ).then_inc(s_prep, 1)
                gpsimd.wait_ge(s_prep, t + 1)
                gpsimd.wait_ge(s_dve, 5 * t + 4)
                if t >= 2:
                    gpsimd.wait_ge(s_bloc[t % 2], 16 * (t // 2))
                gpsimd.trigger_dma(1)

        @block.tensor
        def _(tensor):
            tensor.wait_ge(s_w, 16 * N_INIT_DMA)
            # projection
            for rc in range(R):
                tensor.wait_ge(s_xt[rc % 2], 16 * (rc // 2 + 1))
                if rc >= 2:
                    tensor.wait_ge(s_pc, rc - 1)
                mm = None
                for k in range(KC):
                    mm = tensor.matmul(
                        psps[rc % 2][:, :],
                        xts[rc % 2][:, k * P:(k + 1) * P],
                        wiT_sb[:, k * GS:(k + 1) * GS],
                        start=(k == 0),
                        stop=(k == KC - 1),
                    )
                mm.then_inc(s_pp, 1)
            # recurrence
            for t in range(seq_len):
                tensor.wait_ge(s_xgr[t % 2], 16 * (t // 2 + 1))
                if t >= 1 and comm:
                    tensor.wait_ge(s_hrs[t % 2], 16 * ((t + 1) // 2))
                if t >= 2:
                    tensor.wait_ge(s_act, 3 * (t - 2) + 2)
                mm = None
                for c in range(4):
                    out = psrs[t % 2][:, c * NB:(c + 1) * NB]
                    tensor.matmul(
                        out,
                        xgs[t % 2][:, c * P:(c + 1) * P],
                        eye_sb[:, :],
                        start=True,
                        stop=False,
                    )
                    for j in range(KC):
                        mm = tensor.matmul(
                            out,
                            whT_sb[:, j * GS + c * P: j * GS + (c + 1) * P],
                            hbs[t % 2][:, j * NB:(j + 1) * NB],
                            start=False,
                            stop=(j == KC - 1),
                        )
                mm.then_inc(s_mm, 1)

        @block.scalar
        def _(scalar):
            for t in range(seq_len):
                scalar.wait_ge(s_mm, t + 1)
                if t >= 2:
                    scalar.wait_ge(s_dve, 5 * (t - 2) + 5)
                scalar.activation(
                    gas[t % 2][:, 0:3 * NB], psrs[t % 2][:, 0:3 * NB], SIG
                ).then_inc(s_act, 1)
                scalar.activation(
                    gas[t % 2][:, 3 * NB:4 * NB], psrs[t % 2][:, 3 * NB:4 * NB],
                    TANH,
                ).then_inc(s_act, 1)
                scalar.wait_ge(s_dve, 5 * t + 3)
                scalar.activation(
                    tcs[t % 2][:, :], c_sb[:, :], TANH
                ).then_inc(s_act, 1)

        @block.vector
        def _(vector):
            # projection epilogue: psum + bias -> bf16 staging
            for rc in range(R):
                vector.wait_ge(s_pp, rc + 1)
                if rc >= 2:
                    vector.wait_ge(s_xgw[rc % 2], 16 * (rc // 2))
                vector.tensor_tensor(
                    psts[rc % 2][:, :], psps[rc % 2][:, :], biasb_sb[:, :], ADD
                ).then_inc(s_pc, 1)
            # recurrence: gates order along free dim: [i | f | o | g] x NB
            for t in range(seq_len):
                ga = gas[t % 2]
                vector.wait_ge(s_act, 3 * t + 2)
                vector.tensor_tensor(
                    tmp_ig[:, :], ga[:, 0:NB], ga[:, 3 * NB:4 * NB], MULT
                ).then_inc(s_dve, 1)
                if t >= 1:
                    # same-engine RAW: c_sb written by op3 of step t-1
                    vector.wait_ge(s_dve, 5 * (t - 1) + 3)
                vector.tensor_tensor(
                    c_sb[:, :], ga[:, NB:2 * NB], c_sb[:, :], MULT
                ).then_inc(s_dve, 1)
                # same-engine RAW: tmp_ig (op1) and c_sb (op2) of this step
                vector.wait_ge(s_dve, 5 * t + 2)
                vector.tensor_tensor(
                    c_sb[:, :], c_sb[:, :], tmp_ig[:, :], ADD
                ).then_inc(s_dve, 1)
                vector.wait_ge(s_act, 3 * t + 3)
                if t >= 2 and comm:
                    vector.wait_ge(s_bloc[t % 2], 16 * (t // 2))
                vector.tensor_tensor(
                    hsts[t % 2][:, :], ga[:, 2 * NB:3 * NB], tcs[t % 2][:, :],
                    MULT,
                ).then_inc(s_dve, 1)
                if t >= 2:
                    vector.wait_ge(s_hso[t % 2], 16 * (t // 2))
                vector.tensor_tensor(
                    hfs[t % 2][:, :], ga[:, 2 * NB:3 * NB], tcs[t % 2][:, :],
                    MULT,
                ).then_inc(s_dve, 1)

    nc.compile()
    return nc


def make_in_maps(inputs, seq_len=S):
    x = np.asarray(inputs["x"], np.float32)[:, :seq_len, :]
    h0 = np.asarray(inputs["h0"], np.float32)
    c0 = np.asarray(inputs["c0"], np.float32)
    Wi = np.asarray(inputs["Wi"], np.float32)
    bi = np.asarray(inputs["bi"], np.float32)
    Wh = np.asarray(inputs["Wh"], np.float32)
    bh = np.asarray(inputs["bh"], np.float32)

    # xT[d, t*B + b] = x[b, t, d]
    xT = np.ascontiguousarray(
        x.transpose(2, 1, 0).reshape(D, seq_len * B)
    ).astype(_BF)
    # h0T[p, j*NB + b] = h0[j*HS + p]
    h0T = np.ascontiguousarray(
        np.broadcast_to(
            h0.reshape(NCORES, HS)[:, :, None], (NCORES, HS, NB)
        ).transpose(1, 0, 2).reshape(HS, NCORES * NB)
    ).astype(_BF)
    eye = np.eye(NB, dtype=np.float32).astype(_BF)
    bias = bi + bh

    in_maps = []
    for r in range(NCORES):
        sl = np.arange(r * HS, (r + 1) * HS)
        rows = np.concatenate([sl, H + sl, 3 * H + sl, 2 * H + sl])  # i,f,o,g
        wiT_r = np.ascontiguousarray(Wi[rows].T).astype(_BF)
        whT_r = np.ascontiguousarray(Wh[rows].T).astype(_BF)
        biasb_r = np.ascontiguousarray(
            np.broadcast_to(bias[rows][None, :], (P, GS))
        ).astype(np.float32)
        c0_r = np.ascontiguousarray(
            np.broadcast_to(c0[sl][:, None], (HS, NB))
        ).astype(np.float32)
        in_maps.append({
            "xT": xT, "wiT": wiT_r, "whT": whT_r, "biasb": biasb_r,
            "h0T": h0T, "c0": c0_r, "eye": eye,
        })
    return in_maps


def assemble(results, seq_len=S):
    hidden = np.empty((B, seq_len, H), np.float32)
    cfin = np.empty((B, H), np.float32)
    for r in range(NCORES):
        out = results[r]
        hsT_r = np.asarray(out["hsT"], np.float32).reshape(seq_len, P, NB)
        hidden[:, :, r * HS:(r + 1) * HS] = hsT_r.transpose(2, 0, 1)
        cfin[:, r * HS:(r + 1) * HS] = np.asarray(out["cT"], np.float32).T
    hfin = np.ascontiguousarray(hidden[:, -1, :])
    return hidden, (hfin, cfin)


def kernel(**inputs):
    from concourse import bass_utils

    nc = build_nc(S)
    in_maps = make_in_maps(inputs, S)
    res = bass_utils.run_bass_kernel_spmd(
        nc, in_maps, core_ids=list(range(NCORES))
    )
    return assemble(res.results, S)


if __name__ == "__main__":
    import reference

    inputs = {k: np.asarray(v) for k, v in reference.setup_inputs().items()}
    out = kernel(**inputs)
    print("kernel ran; hidden_seq shape:", out[0].shape)


# revision 9
# speedup vs baseline: 1.4596x; 1.4596x over previous
"""Trainium2 Bass kernel for nn_CustomLSTM (B=64, S=512, D=H=1024) on 8 cores.

Sharding: hidden-dimension split (128 hidden units / core), full batch on
every core.  Per timestep each core computes its 512 gate rows
(i,f,o,g x 128) as [128 gates, 64 batch] tiles (hidden-major), updates its
c/h slices, and broadcasts its h slice [128, 64] bf16 to all 8 cores'
SBUF via remote_dma_broadcast (slot = sender core id).  The input
projection xg = x @ Wi.T + (bi+bh) is computed on-device in a first phase
(each core computes its own 512 gate columns for all (t, b)) and streamed
from DRAM during the recurrence, folded into PSUM with an identity-matmul
transpose-accumulate.

kernel(**inputs) takes the full unsharded inputs and returns
(hidden_seq, (h_final, c_final)) matching the reference.
"""

import numpy as np
import ml_dtypes

B, S, D, H = 64, 512, 1024, 1024
NCORES = 8
HS = H // NCORES      # 128 hidden units per core
GS = 4 * HS           # 512 gate rows per core
P = 128               # partitions
NB = B                # 64 batch columns per tile
KC = D // P           # 8 contraction chunks

_BF = ml_dtypes.bfloat16


def build_nc(seq_len=S, comm=True):
    import concourse.bass as bass
    import concourse.mybir as mybir
    from concourse import bacc
    from contextlib import ExitStack

    f32 = mybir.dt.float32
    bf16 = mybir.dt.bfloat16
    SIG = mybir.ActivationFunctionType.Sigmoid
    TANH = mybir.ActivationFunctionType.Tanh
    MULT = mybir.AluOpType.mult
    ADD = mybir.AluOpType.add

    R = seq_len * B // P  # projection row-chunks (128 (t,b) rows each)
    assert seq_len >= 4 and seq_len % 2 == 0

    nc = bacc.Bacc("TRN2", target_bir_lowering=False, debug=False,
                   num_devices=NCORES)

    # ---- DRAM ----
    xT = nc.dram_tensor("xT", [D, seq_len * B], bf16, kind="ExternalInput")
    wiT = nc.dram_tensor("wiT", [D, GS], bf16, kind="ExternalInput")
    whT = nc.dram_tensor("whT", [D, GS], bf16, kind="ExternalInput")
    biasb = nc.dram_tensor("biasb", [P, GS], f32, kind="ExternalInput")
    h0T = nc.dram_tensor("h0T", [P, NCORES * NB], bf16, kind="ExternalInput")
    c0 = nc.dram_tensor("c0", [P, NB], f32, kind="ExternalInput")
    eye = nc.dram_tensor("eye", [NB, NB], bf16, kind="ExternalInput")
    hsT = nc.dram_tensor("hsT", [seq_len * P, NB], f32, kind="ExternalOutput")
    cT = nc.dram_tensor("cT", [P, NB], f32, kind="ExternalOutput")
    xg = nc.dram_tensor("xg", [seq_len * B, GS], bf16)  # internal

    es = ExitStack()
    with es:
        # ---- SBUF ----
        whT_sb = es.enter_context(nc.sbuf_tensor("whT_sb", [P, KC * GS], bf16))
        wiT_sb = es.enter_context(nc.sbuf_tensor("wiT_sb", [P, KC * GS], bf16))
        biasb_sb = es.enter_context(nc.sbuf_tensor("biasb_sb", [P, GS], f32))
        xt0 = es.enter_context(nc.sbuf_tensor("xt0", [P, KC * P], bf16))
        xt1 = es.enter_context(nc.sbuf_tensor("xt1", [P, KC * P], bf16))
        pst0 = es.enter_context(nc.sbuf_tensor("pst0", [P, GS], bf16))
        pst1 = es.enter_context(nc.sbuf_tensor("pst1", [P, GS], bf16))
        xg0 = es.enter_context(nc.sbuf_tensor("xg0", [NB, GS], bf16))
        xg1 = es.enter_context(nc.sbuf_tensor("xg1", [NB, GS], bf16))
        hb0 = es.enter_context(nc.sbuf_tensor("hb0", [P, NCORES * NB], bf16))
        hb1 = es.enter_context(nc.sbuf_tensor("hb1", [P, NCORES * NB], bf16))
        hst0 = es.enter_context(nc.sbuf_tensor("hst0", [P, NB], bf16))
        hst1 = es.enter_context(nc.sbuf_tensor("hst1", [P, NB], bf16))
        hf0 = es.enter_context(nc.sbuf_tensor("hf0", [P, NB], f32))
        hf1 = es.enter_context(nc.sbuf_tensor("hf1", [P, NB], f32))
        ga0 = es.enter_context(nc.sbuf_tensor("ga0", [P, 4 * NB], f32))
        ga1 = es.enter_context(nc.sbuf_tensor("ga1", [P, 4 * NB], f32))
        tc0 = es.enter_context(nc.sbuf_tensor("tc0", [P, NB], f32))
        tc1 = es.enter_context(nc.sbuf_tensor("tc1", [P, NB], f32))
        tmp_ig = es.enter_context(nc.sbuf_tensor("tmp_ig", [P, NB], f32))
        c_sb = es.enter_context(nc.sbuf_tensor("c_sb", [P, NB], f32))
        eye_sb = es.enter_context(nc.sbuf_tensor("eye_sb", [NB, NB], bf16))
        # ---- PSUM ----
        psp0 = es.enter_context(nc.psum_tensor("psp0", [P, GS], f32))
        psp1 = es.enter_context(nc.psum_tensor("psp1", [P, GS], f32))
        psr0 = es.enter_context(nc.psum_tensor("psr0", [P, 4 * NB], f32))
        psr1 = es.enter_context(nc.psum_tensor("psr1", [P, 4 * NB], f32))

        xts = [xt0, xt1]
        psts = [pst0, pst1]
        xgs = [xg0, xg1]
        hbs = [hb0, hb1]
        hsts = [hst0, hst1]
        hfs = [hf0, hf1]
        gas = [ga0, ga1]
        tcs = [tc0, tc1]
        psps = [psp0, psp1]
        psrs = [psr0, psr1]

        # ---- semaphores ----
        s_w = es.enter_context(nc.semaphore("s_w"))
        s_xt = [es.enter_context(nc.semaphore(f"s_xt{i}")) for i in range(2)]
        s_pp = es.enter_context(nc.semaphore("s_pp"))
        s_pc = es.enter_context(nc.semaphore("s_pc"))
        s_xgw = [es.enter_context(nc.semaphore(f"s_xgw{i}")) for i in range(2)]
        s_xgr = [es.enter_context(nc.semaphore(f"s_xgr{i}")) for i in range(2)]
        s_hrs = [es.enter_context(nc.semaphore(f"s_hr{i}")) for i in range(2)]
        s_bloc = [es.enter_context(nc.semaphore(f"s_bloc{i}")) for i in range(2)]
        s_prep = es.enter_context(nc.semaphore("s_prep"))
        s_mm = es.enter_context(nc.semaphore("s_mm"))
        s_act = es.enter_context(nc.semaphore("s_act"))
        s_dve = es.enter_context(nc.semaphore("s_dve"))
        s_hso = [es.enter_context(nc.semaphore(f"s_hso{i}")) for i in range(2)]
        s_fin = es.enter_context(nc.semaphore("s_fin"))

        N_INIT_DMA = 6  # whT, wiT, biasb, h0T, c0, eye
        block = es.enter_context(nc.Block())

        @block.sync
        def _(sync):
            # init loads
            sync.dma_start(
                whT_sb[:, :].rearrange("p (c g) -> p c g", c=KC),
                whT[:, :].rearrange("(c p) g -> p c g", p=P),
            ).then_inc(s_w, 16)
            sync.dma_start(
                wiT_sb[:, :].rearrange("p (c g) -> p c g", c=KC),
                wiT[:, :].rearrange("(c p) g -> p c g", p=P),
            ).then_inc(s_w, 16)
            sync.dma_start(biasb_sb[:, :], biasb[:, :]).then_inc(s_w, 16)
            sync.dma_start(hb0[:, :], h0T[:, :]).then_inc(s_w, 16)
            sync.dma_start(c_sb[:, :], c0[:, :]).then_inc(s_w, 16)
            sync.dma_start(eye_sb[:, :], eye[:, :]).then_inc(s_w, 16)

            # projection xT prologue (row-chunks 0, 1)
            for i in range(2):
                sync.dma_start(
                    xts[i][:, :].rearrange("p (c r) -> p c r", c=KC),
                    xT[:, i * P:(i + 1) * P].rearrange("(c p) r -> p c r", p=P),
                ).then_inc(s_xt[i], 16)
            # projection loop
            for rc in range(R):
                sync.wait_ge(s_pc, rc + 1)
                sync.dma_start(
                    xg[rc * P:(rc + 1) * P, :], psts[rc % 2][:, :]
                ).then_inc(s_xgw[rc % 2], 16)
                if rc + 2 < R:
                    sync.wait_ge(s_pp, rc + 1)
                    sync.dma_start(
                        xts[rc % 2][:, :].rearrange("p (c r) -> p c r", c=KC),
                        xT[:, (rc + 2) * P:(rc + 3) * P].rearrange(
                            "(c p) r -> p c r", p=P
                        ),
                    ).then_inc(s_xt[rc % 2], 16)
            # recurrence xg prologue (t = 0, 1)
            for t in range(2):
                rcs = t // 2  # source row-chunk
                sync.wait_ge(s_xgw[rcs % 2], 16 * (rcs // 2 + 1))
                sync.dma_start(
                    xgs[t % 2][:, :], xg[t * B:(t + 1) * B, :]
                ).then_inc(s_xgr[t % 2], 16)
            # recurrence loop
            for t in range(seq_len):
                sync.wait_ge(s_dve, 5 * t + 5)
                sync.dma_start(
                    hsT[t * P:(t + 1) * P, :], hfs[t % 2][:, :]
                ).then_inc(s_hso[t % 2], 16)
                if t + 2 < seq_len:
                    rcs = (t + 2) // 2
                    sync.wait_ge(s_xgw[rcs % 2], 16 * (rcs // 2 + 1))
                    sync.wait_ge(s_mm, t + 1)
                    sync.dma_start(
                        xgs[t % 2][:, :], xg[(t + 2) * B:(t + 3) * B, :]
                    ).then_inc(s_xgr[t % 2], 16)
            # final c
            sync.wait_ge(s_dve, 5 * seq_len)
            sync.dma_start(cT[:, :], c_sb[:, :]).then_inc(s_fin, 16)
            sync.wait_ge(s_fin, 16)

        @block.gpsimd
        def _(gpsimd: bass.BassGpSimd):
            if not comm:
                return
            pid = gpsimd.partition_id()
            for t in range(seq_len):
                hbn = hbs[(t + 1) % 2]
                gpsimd.remote_dma_broadcast(
                    out_ap=hbn[:, bass.ts(pid, NB)],
                    in_ap=hsts[t % 2][:, :],
                    remote_sem=s_hrs[(t + 1) % 2],
                    local_sem=s_bloc[t % 2],
                    rdests=[(0, k) for k in range(NCORES)],
                ).then_inc(s_prep, 1)
                gpsimd.wait_ge(s_prep, t + 1)
                gpsimd.wait_ge(s_dve, 5 * t + 4)
                if t >= 2:
                    gpsimd.wait_ge(s_bloc[t % 2], 16 * (t // 2))
                gpsimd.trigger_dma(1)

        @block.tensor
        def _(tensor):
            tensor.wait_ge(s_w, 16 * N_INIT_DMA)
            # projection
            for rc in range(R):
                tensor.wait_ge(s_xt[rc % 2], 16 * (rc // 2 + 1))
                if rc >= 2:
                    tensor.wait_ge(s_pc, rc - 1)
                mm = None
                for k in range(KC):
                    mm = tensor.matmul(
                        psps[rc % 2][:, :],
                        xts[rc % 2][:, k * P:(k + 1) * P],
                        wiT_sb[:, k * GS:(k + 1) * GS],
                        start=(k == 0),
                        stop=(k == KC - 1),
                    )
                mm.then_inc(s_pp, 1)
            # recurrence
            for t in range(seq_len):
                tensor.wait_ge(s_xgr[t % 2], 16 * (t // 2 + 1))
                if t >= 1 and comm:
                    tensor.wait_ge(s_hrs[t % 2], 16 * ((t + 1) // 2))
                if t >= 2:
                    tensor.wait_ge(s_act, 4 * (t - 2) + 3)
                mm = None
                for c in range(4):
                    out = psrs[t % 2][:, c * NB:(c + 1) * NB]
                    tensor.matmul(
                        out,
                        xgs[t % 2][:, c * P:(c + 1) * P],
                        eye_sb[:, :],
                        start=True,
                        stop=False,
                    )
                    for j in range(KC):
                        mm = tensor.matmul(
                            out,
                            whT_sb[:, j * GS + c * P: j * GS + (c + 1) * P],
                            hbs[t % 2][:, j * NB:(j + 1) * NB],
                            start=False,
                            stop=(j == KC - 1),
                        )
                mm.then_inc(s_mm, 1)

        @block.scalar
        def _(scalar):
            for t in range(seq_len):
                scalar.wait_ge(s_mm, t + 1)
                if t >= 2:
                    scalar.wait_ge(s_dve, 5 * (t - 2) + 5)
                # i,f first so DVE's c-update can start before o is done
                scalar.activation(
                    gas[t % 2][:, 0:2 * NB], psrs[t % 2][:, 0:2 * NB], SIG
                ).then_inc(s_act, 1)
                scalar.activation(
                    gas[t % 2][:, 3 * NB:4 * NB], psrs[t % 2][:, 3 * NB:4 * NB],
                    TANH,
                ).then_inc(s_act, 1)
                scalar.activation(
                    gas[t % 2][:, 2 * NB:3 * NB], psrs[t % 2][:, 2 * NB:3 * NB],
                    SIG,
                ).then_inc(s_act, 1)
                scalar.wait_ge(s_dve, 5 * t + 3)
                scalar.activation(
                    tcs[t % 2][:, :], c_sb[:, :], TANH
                ).then_inc(s_act, 1)

        @block.vector
        def _(vector):
            # projection epilogue: psum + bias -> bf16 staging
            for rc in range(R):
                vector.wait_ge(s_pp, rc + 1)
                if rc >= 2:
                    vector.wait_ge(s_xgw[rc % 2], 16 * (rc // 2))
                vector.tensor_tensor(
                    psts[rc % 2][:, :], psps[rc % 2][:, :], biasb_sb[:, :], ADD
                ).then_inc(s_pc, 1)
            # recurrence: gates order along free dim: [i | f | o | g] x NB
            for t in range(seq_len):
                ga = gas[t % 2]
                vector.wait_ge(s_act, 4 * t + 2)
                vector.tensor_tensor(
                    tmp_ig[:, :], ga[:, 0:NB], ga[:, 3 * NB:4 * NB], MULT
                ).then_inc(s_dve, 1)
                if t >= 1:
                    # same-engine RAW: c_sb written by op3 of step t-1
                    vector.wait_ge(s_dve, 5 * (t - 1) + 3)
                vector.tensor_tensor(
                    c_sb[:, :], ga[:, NB:2 * NB], c_sb[:, :], MULT
                ).then_inc(s_dve, 1)
                # same-engine RAW: tmp_ig (op1) and c_sb (op2) of this step
                vector.wait_ge(s_dve, 5 * t + 2)
                vector.tensor_tensor(
                    c_sb[:, :], c_sb[:, :], tmp_ig[:, :], ADD
                ).then_inc(s_dve, 1)
                vector.wait_ge(s_act, 4 * t + 4)
                if t >= 2 and comm:
                    vector.wait_ge(s_bloc[t % 2], 16 * (t // 2))
                vector.tensor_tensor(
                    hsts[t % 2][:, :], ga[:, 2 * NB:3 * NB], tcs[t % 2][:, :],
                    MULT,
                ).then_inc(s_dve, 1)
                if t >= 2:
                    vector.wait_ge(s_hso[t % 2], 16 * (t // 2))
                vector.tensor_tensor(
                    hfs[t % 2][:, :], ga[:, 2 * NB:3 * NB], tcs[t % 2][:, :],
                    MULT,
                ).then_inc(s_dve, 1)

    nc.compile()
    return nc


def make_in_maps(inputs, seq_len=S):
    x = np.asarray(inputs["x"], np.float32)[:, :seq_len, :]
    h0 = np.asarray(inputs["h0"], np.float32)
    c0 = np.asarray(inputs["c0"], np.float32)
    Wi = np.asarray(inputs["Wi"], np.float32)
    bi = np.asarray(inputs["bi"], np.float32)
    Wh = np.asarray(inputs["Wh"], np.float32)
    bh = np.asarray(inputs["bh"], np.float32)

    # xT[d, t*B + b] = x[b, t, d]
    xT = np.ascontiguousarray(
        x.transpose(2, 1, 0).reshape(D, seq_len * B)
    ).astype(_BF)
    # h0T[p, j*NB + b] = h0[j*HS + p]
    h0T = np.ascontiguousarray(
        np.broadcast_to(
            h0.reshape(NCORES, HS)[:, :, None], (NCORES, HS, NB)
        ).transpose(1, 0, 2).reshape(HS, NCORES * NB)
    ).astype(_BF)
    eye = np.eye(NB, dtype=np.float32).astype(_BF)
    bias = bi + bh

    in_maps = []
    for r in range(NCORES):
        sl = np.arange(r * HS, (r + 1) * HS)
        rows = np.concatenate([sl, H + sl, 3 * H + sl, 2 * H + sl])  # i,f,o,g
        wiT_r = np.ascontiguousarray(Wi[rows].T).astype(_BF)
        whT_r = np.ascontiguousarray(Wh[rows].T).astype(_BF)
        biasb_r = np.ascontiguousarray(
            np.broadcast_to(bias[rows][None, :], (P, GS))
        ).astype(np.float32)
        c0_r = np.ascontiguousarray(
            np.broadcast_to(c0[sl][:, None], (HS, NB))
        ).astype(np.float32)
        in_maps.append({
            "xT": xT, "wiT": wiT_r, "whT": whT_r, "biasb": biasb_r,
            "h0T": h0T, "c0": c0_r, "eye": eye,
        })
    return in_maps


def assemble(results, seq_len=S):
    hidden = np.empty((B, seq_len, H), np.float32)
    cfin = np.empty((B, H), np.float32)
    for r in range(NCORES):
        out = results[r]
        hsT_r = np.asarray(out["hsT"], np.float32).reshape(seq_len, P, NB)
        hidden[:, :, r * HS:(r + 1) * HS] = hsT_r.transpose(2, 0, 1)
        cfin[:, r * HS:(r + 1) * HS] = np.asarray(out["cT"], np.float32).T
    hfin = np.ascontiguousarray(hidden[:, -1, :])
    return hidden, (hfin, cfin)


def kernel(**inputs):
    from concourse import bass_utils

    nc = build_nc(S)
    in_maps = make_in_maps(inputs, S)
    res = bass_utils.run_bass_kernel_spmd(
        nc, in_maps, core_ids=list(range(NCORES))
    )
    return assemble(res.results, S)


if __name__ == "__main__":
    import reference

    inputs = {k: np.asarray(v) for k, v in reference.setup_inputs().items()}
    out = kernel(**inputs)
    print("kernel ran; hidden_seq shape:", out[0].shape)
